# revision 2
# baseline (speedup 1.0000x reference)
"""Multi-head attention kernel for Trainium2, 8 NeuronCores.

Problem: B=4, S=2048, D=1024, H=16 heads, d_k=64 (fp32).
    out = softmax((Q Wq + bq)(K Wk + bk)^T / 8) (V Wv + bv) Wo + bo

Sharding: core c handles batch b = c//2 and head-group g = c%2
(8 heads, a 512-wide slice of the model dim). W_q/W_k/W_v are split
column-wise, W_o row-wise; each core computes a full [2048, 1024]
partial output and the host sums core pairs and adds bo.

Per-core dataflow (everything fp32):
  1. PE-transpose X (the relevant input) into X^T slices (in-dim on
     partitions) per 512-column sequence chunk.
  2. Projections: q^T, k^T produced transposed (head-dim on partitions,
     bias via per-partition tensor_scalar add; 1/sqrt(d_k) folded into
     Wq/bq on the host); v produced in natural orientation with bias
     via a ones-row matmul, stored ones-augmented ([v | 1] per 128-row
     chunk) so the attn@V matmul also produces softmax denominators.
  3. Attention per head: S^T tiles = k^T.T @ q^T (keys on partitions),
     exp on ACT straight out of PSUM (no max subtraction needed:
     scores ~ N(0,1), fp32 exp overflows only beyond 88), then
     O^T = [v|1].T @ exp(S^T) accumulated over key chunks; row 64 of
     the [65, 512] result is the softmax denominator.
  4. Normalize O^T by reciprocal denominators (broadcast across
     partitions via a DRAM round-trip DMA), then the output projection
     contracts the packed O^T tiles against Wo rows.
"""

import sys

sys.path.insert(0, '/opt/trn_rl_repo')

import numpy as np

B = 4
S = 2048
D = 1024
H = 16
DK = 64
HPC = 8          # heads per core
DH = 512         # model-dim slice per core (HPC * DK)
N_CORES = 8
SC = 512         # sequence chunk for projections
NSC = S // SC    # 4
NKB = S // 128   # 16 key blocks
NQC = S // 512   # 4 query chunks

_CACHE = {}


def _build():
    import concourse.bass as bass
    import concourse.tile as tile
    from concourse import mybir
    from concourse.masks import make_identity
    import bass_rust

    # ---- workarounds for this walrus build: max ONE sync wait/instr ----
    def _patched_drain_and_barrier(self, tick_clock, wait_clock):
        drain_inst = self.nc.sync.drain()
        wait_clock.add_sem_waits(
            drain_inst.ins, tile.ScopedClock({None: tick_clock.global_clock}))
        mi = drain_inst.ins
        si = mi.sync_info
        waits = list(si.on_wait or []) if si is not None else []
        if len(waits) > 1:
            si.on_wait = waits[:1]
            for w in waits[1:]:
                d2 = self.nc.sync.drain()
                si2 = d2.ins.sync_info
                if si2 is None:
                    d2.ins.sync_info = bass_rust.SyncInfo(on_wait=[w], on_update=[])
                else:
                    si2.on_wait = [w]
        self.nc.all_engine_barrier()
        popped = self.nc._tile_sem_poison_stack.pop()
        assert popped is self._sem_poison
        self.nc.clear_and_free_semaphores(list(self.sems.allocated().values()))
        self.nc.all_engine_barrier()

    tile.TileContext._drain_and_barrier = _patched_drain_and_barrier

    def legalize_sync_waits(nc):
        for f in nc.m.functions:
            for bb in f.blocks:
                il = bb.instructions
                if not any(
                    inst.sync_info is not None
                    and len(inst.sync_info.on_wait or []) > 1
                    for inst in il
                ):
                    continue
                new = []
                for inst in il:
                    si = inst.sync_info
                    waits = list(si.on_wait or []) if si is not None else []
                    if len(waits) > 1 and inst.engine != mybir.EngineType.Unassigned:
                        eng = nc.engines[inst.engine]
                        for w in waits[:-1]:
                            nop = eng.nop()
                            nopmi = nop.ins
                            cur = nc.cur_bb.bb if hasattr(nc.cur_bb, 'bb') else nc.cur_bb
                            cil = cur.instructions
                            for k in range(len(cil) - 1, -1, -1):
                                if cil[k].name == nopmi.name:
                                    del cil[k]
                                    break
                            si2 = nopmi.sync_info
                            if si2 is None:
                                nopmi.sync_info = bass_rust.SyncInfo(
                                    on_wait=[w], on_update=[])
                            else:
                                si2.on_wait = [w]
                            new.append(nopmi)
                        si.on_wait = waits[-1:]
                    new.append(inst)
                il[:] = new

    F32 = mybir.dt.float32
    nc = bass.Bass('TRN2', target_bir_lowering=False, debug=False)

    xq = nc.dram_tensor('xq', [S, D], F32, kind='ExternalInput').ap()
    xk = nc.dram_tensor('xk', [S, D], F32, kind='ExternalInput').ap()
    xv = nc.dram_tensor('xv', [S, D], F32, kind='ExternalInput').ap()
    wq = nc.dram_tensor('wq', [D, DH], F32, kind='ExternalInput').ap()
    wk = nc.dram_tensor('wk', [D, DH], F32, kind='ExternalInput').ap()
    wv = nc.dram_tensor('wv', [D, DH], F32, kind='ExternalInput').ap()
    bq = nc.dram_tensor('bq', [128, 4], F32, kind='ExternalInput').ap()
    bk = nc.dram_tensor('bk', [128, 4], F32, kind='ExternalInput').ap()
    bv = nc.dram_tensor('bv', [1, DH], F32, kind='ExternalInput').ap()
    wo = nc.dram_tensor('wo', [DH, D], F32, kind='ExternalInput').ap()
    out = nc.dram_tensor('out', [S, D], F32, kind='ExternalOutput').ap()

    EXP = mybir.ActivationFunctionType.Exp

    with tile.TileContext(nc) as tc:
        with tc.tile_pool(name='const', bufs=1) as constp, \
             tc.tile_pool(name='wp', bufs=1) as wp, \
             tc.tile_pool(name='xp', bufs=2) as xp, \
             tc.tile_pool(name='xtp', bufs=1) as xtp, \
             tc.tile_pool(name='qkv', bufs=1) as qkv, \
             tc.tile_pool(name='atp', bufs=3) as atp, \
             tc.tile_pool(name='bst', bufs=3) as bst, \
             tc.tile_pool(name='drp', bufs=1, space='DRAM') as drp, \
             tc.tile_pool(name='pa', bufs=2, space='PSUM') as pa, \
             tc.tile_pool(name='pb', bufs=4, space='PSUM') as pb:

            ident = constp.tile([128, 128], F32, name='ident')
            make_identity(nc, ident)
            ones1 = constp.tile([1, 128], F32, name='ones1')
            nc.vector.memset(ones1[:], 1.0)
            bq_t = constp.tile([128, 4], F32, name='bq_t')
            bk_t = constp.tile([128, 4], F32, name='bk_t')
            bv_t = constp.tile([1, DH], F32, name='bv_t')
            nc.sync.dma_start(bq_t[:], bq[:])
            nc.sync.dma_start(bk_t[:], bk[:])
            nc.sync.dma_start(bv_t[:], bv[:])
            # head h's denominators live at partition (h//2)*32 + h%2
            # (DVE partition offsets must be 32-aligned)
            sums = constp.tile([128, S], F32, name='sums')
            scratch = drp.tile([HPC, S], F32, name='scratch')

            # persistent activation tiles
            qT = [qkv.tile([128, S], F32, name=f'qT{j}', tag=f'qT{j}')
                  for j in range(4)]
            kT = [qkv.tile([128, S], F32, name=f'kT{j}', tag=f'kT{j}')
                  for j in range(4)]
            OT = [qkv.tile([128, S], F32, name=f'OT{j}', tag=f'OT{j}')
                  for j in range(4)]
            v_aug = qkv.tile([128, HPC * NKB * 65], F32, name='v_aug',
                             tag='v_aug')
            # ones columns of v_aug (col 64 of each 65-wide chunk)
            v_view = v_aug.rearrange('p (h c w) -> p h c w', h=HPC, c=NKB)
            nc.vector.memset(v_view[:, :, :, 64:65], 1.0)

            # ---------------- projections ----------------
            w_dram = {0: wq, 1: wk, 2: wv}
            for pi in range(3):
                w_t = wp.tile([128, 8 * DH], F32, name=f'w_t{pi}', tag='w')
                nc.sync.dma_start(
                    w_t.rearrange('p (c n) -> p c n', c=8),
                    w_dram[pi].rearrange('(c p) n -> p c n', p=128))
                xsrc = {0: xq, 1: xk, 2: xv}[pi]
                for sc in range(NSC):
                    xt = xtp.tile([128, 8 * SC], F32, name='xt', tag='xt')
                    xtv = xt.rearrange('p (c n) -> p c n', c=8)
                    for rb in range(4):
                        xb = xp.tile([128, D], F32, name='xb', tag='x')
                        r0 = (sc * 4 + rb) * 128
                        nc.sync.dma_start(xb[:], xsrc[r0:r0 + 128, :])
                        for half in range(2):
                            pt = pb.tile([128, 512], F32, name='pt', tag='pb')
                            for qd in range(4):
                                dc = half * 4 + qd
                                nc.tensor.transpose(
                                    pt[:, qd * 128:(qd + 1) * 128],
                                    xb[:, dc * 128:(dc + 1) * 128], ident[:])
                            nc.vector.tensor_copy(
                                xtv[:, half * 4:(half + 1) * 4,
                                    rb * 128:(rb + 1) * 128],
                                pt.rearrange('p (c n) -> p c n', c=4))
                    if pi < 2:
                        dst = qT if pi == 0 else kT
                        bias = bq_t if pi == 0 else bk_t
                        for j in range(4):
                            acc = pb.tile([128, 512], F32, name='acc', tag='pb')
                            for kc in range(8):
                                nc.tensor.matmul(
                                    acc[:],
                                    w_t[:, kc * DH + j * 128:
                                        kc * DH + (j + 1) * 128],
                                    xtv[:, kc, :],
                                    start=(kc == 0), stop=(kc == 7))
                            nc.vector.tensor_scalar_add(
                                dst[j][:, sc * SC:(sc + 1) * SC], acc[:],
                                bias[:, j:j + 1])
                    else:
                        for rb in range(4):
                            acc = pb.tile([128, 512], F32, name='acc', tag='pb')
                            for kc in range(8):
                                nc.tensor.matmul(
                                    acc[:],
                                    xtv[:, kc, rb * 128:(rb + 1) * 128],
                                    w_t[:, kc * DH:(kc + 1) * DH],
                                    start=(kc == 0), stop=False)
                            nc.tensor.matmul(
                                acc[:], ones1[0:1, :], bv_t[0:1, :],
                                start=False, stop=True)
                            cg = sc * 4 + rb
                            nc.vector.tensor_copy(
                                v_view[:, :, cg, 0:64],
                                acc.rearrange('p (h d) -> p h d', h=HPC))

            # ---------------- attention ----------------
            for j in range(4):
                for hi in range(2):
                    h = 2 * j + hi
                    po = hi * 64
                    for qcg in range(2):
                        accs = []
                        for qh in range(2):
                            a = pb.tile([128, 512], F32, name='acc_b', tag='pb')
                            accs.append(a)
                        for kb in range(NKB):
                            pw = pa.tile([128, 1024], F32, name='pw', tag='pa')
                            for qh in range(2):
                                nc.tensor.matmul(
                                    pw[:, qh * 512:(qh + 1) * 512],
                                    kT[j][po:po + 64, kb * 128:(kb + 1) * 128],
                                    qT[j][po:po + 64,
                                          qcg * 1024 + qh * 512:
                                          qcg * 1024 + (qh + 1) * 512],
                                    start=True, stop=True)
                            at = atp.tile([128, 1024], F32, name='at', tag='at')
                            nc.scalar.activation(at[:], pw[:], EXP)
                            for qh in range(2):
                                nc.tensor.matmul(
                                    accs[qh][0:65, :],
                                    v_aug[:, (h * NKB + kb) * 65:
                                          (h * NKB + kb) * 65 + 65],
                                    at[:, qh * 512:(qh + 1) * 512],
                                    start=(kb == 0), stop=(kb == NKB - 1))
                        for qh in range(2):
                            qc = qcg * 2 + qh
                            st = bst.tile([65, 512], F32, name='st', tag='bst')
                            nc.vector.tensor_copy(st[0:65, :], accs[qh][0:65, :])
                            nc.sync.dma_start(
                                OT[j][po:po + 64, qc * 512:(qc + 1) * 512],
                                st[0:64, :])
                            srow = j * 32 + hi
                            nc.sync.dma_start(
                                sums[srow:srow + 1, qc * 512:(qc + 1) * 512],
                                st[64:65, :])
                # normalize this pair's O^T
                nc.vector.reciprocal(sums[32 * j:32 * j + 2, :],
                                     sums[32 * j:32 * j + 2, :])
                nc.sync.dma_start(scratch[2 * j:2 * j + 2, :],
                                  sums[32 * j:32 * j + 2, :])
                for half in range(2):
                    sc_t = atp.tile([128, 1024], F32, name='sc_t', tag='at')
                    nc.sync.dma_start(
                        sc_t[0:64, :],
                        scratch[2 * j:2 * j + 1,
                                half * 1024:(half + 1) * 1024]
                        .partition_broadcast(64))
                    nc.sync.dma_start(
                        sc_t[64:128, :],
                        scratch[2 * j + 1:2 * j + 2,
                                half * 1024:(half + 1) * 1024]
                        .partition_broadcast(64))
                    nc.vector.tensor_mul(
                        OT[j][:, half * 1024:(half + 1) * 1024],
                        OT[j][:, half * 1024:(half + 1) * 1024], sc_t[:])

            # ---------------- output projection ----------------
            wo_t = xtp.tile([128, 4 * D], F32, name='wo_t', tag='xt')
            nc.sync.dma_start(
                wo_t.rearrange('p (c n) -> p c n', c=4),
                wo.rearrange('(c p) n -> p c n', p=128))
            for qb in range(16):
                oev = atp.tile([128, D], F32, name='oev', tag='at')
                for nco in range(2):
                    acc = pb.tile([128, 512], F32, name='acc_o', tag='pb')
                    for dc in range(4):
                        nc.tensor.matmul(
                            acc[:],
                            OT[dc][:, qb * 128:(qb + 1) * 128],
                            wo_t[:, dc * D + nco * 512:dc * D + (nco + 1) * 512],
                            start=(dc == 0), stop=(dc == 3))
                    nc.vector.tensor_copy(oev[:, nco * 512:(nco + 1) * 512],
                                          acc[:])
                nc.sync.dma_start(out[qb * 128:(qb + 1) * 128, :], oev[:])

    legalize_sync_waits(nc)
    return nc


def _get_nc():
    if 'nc' not in _CACHE:
        _CACHE['nc'] = _build()
    return _CACHE['nc']


def _make_in_maps(Q, K, V, Wq, bq, Wk, bk, Wv, bv, Wo):
    f32 = np.float32
    Q = np.asarray(Q, f32)
    K = np.asarray(K, f32)
    V = np.asarray(V, f32)
    Wq = np.asarray(Wq, f32)
    Wk = np.asarray(Wk, f32)
    Wv = np.asarray(Wv, f32)
    Wo = np.asarray(Wo, f32)
    bq = np.asarray(bq, f32)
    bk = np.asarray(bk, f32)
    bv = np.asarray(bv, f32)
    scale = f32(1.0 / np.sqrt(DK))
    in_maps = []
    for c in range(N_CORES):
        b, g = c // 2, c % 2
        cs = slice(g * DH, (g + 1) * DH)
        in_maps.append({
            'xq': np.ascontiguousarray(Q[b]),
            'xk': np.ascontiguousarray(K[b]),
            'xv': np.ascontiguousarray(V[b]),
            'wq': np.ascontiguousarray(Wq[:, cs] * scale),
            'wk': np.ascontiguousarray(Wk[:, cs]),
            'wv': np.ascontiguousarray(Wv[:, cs]),
            'bq': np.ascontiguousarray((bq[cs] * scale).reshape(4, 128).T),
            'bk': np.ascontiguousarray(bk[cs].reshape(4, 128).T),
            'bv': np.ascontiguousarray(bv[cs].reshape(1, DH)),
            'wo': np.ascontiguousarray(Wo[cs, :]),
        })
    return in_maps


def _run(in_maps, trace=False, tmpdir=None):
    from concourse import bass_utils
    nc = _get_nc()
    kw = {}
    if trace:
        kw = dict(trace=True, tmpdir=tmpdir)
    return bass_utils.run_bass_kernel_spmd(
        nc, in_maps, core_ids=list(range(N_CORES)), **kw)


def kernel(Q, K, V, Wq, bq, Wk, bk, Wv, bv, Wo, bo):
    in_maps = _make_in_maps(Q, K, V, Wq, bq, Wk, bk, Wv, bv, Wo)
    res = _run(in_maps)
    bo = np.asarray(bo, np.float32)
    outs = [r['out'] for r in res.results]
    full = np.stack(
        [outs[2 * b] + outs[2 * b + 1] + bo[None, :] for b in range(B)], axis=0)
    return full.astype(np.float32)
